# revision 25
# baseline (speedup 1.0000x reference)
# Trainium2 Bass kernel for: ConvTranspose2d(64->128, k=4, stride=1) -> spatial
# mean -> +biases -> 10*logsumexp over channels.
#
# Math: with full (K-1) output padding, the mean over the ENTIRE conv-transpose
# output spatial extent sees every input pixel through all K*K taps, so
#   pooled[n,co] = (sum_hw x[n,ci,hw]) @ (sum_kk w[ci,co,kk]) / (Ho*Wo) + cb + eb
# exactly. The conv collapses to a spatial sum + a (Cin x Cout) matmul.
#
# Sharding: data-parallel over batch N=32 across 8 cores (4 batches/core).
# The (Cin,Cout) tap-sum of the replicated weight is precomputed on the host
# (param preprocessing, like weight repacking), so each core only streams its
# 4 MiB x-slice plus one ~134 KiB packed param tensor.
#
# Per-core dataflow (trace-driven, see test.py profiling):
# - x arrives as [256, 4096] (row = (n,ci)); the stream is HBM-bound at
#   ~350 GB/s. All chunks are 1024 cols (4 KiB descriptors) — measured: sub-
#   4KiB descriptors cost 1.5-2us of aggregate stream time.
# - Ring split is deliberately UNEQUAL: the ACT ring (qScalarDynamicHW)
#   carries 1 chunk per row block, the SP ring (qSyncDynamicHW) 3. Rings
#   drain at equal packet rates, so the ACT ring finishes ~6us early and the
#   ACT engine is idle well before the stream ends.
# - Reducers: ACT handles its ring's chunks via activation(Copy, accum_out);
#   DVE tensor_reduce handles the SP ring. The FINAL chunk of each row block
#   is reduced SPLIT: DVE takes cols 0:640, idle ACT takes 640:1024 in
#   parallel, so only ~0.9us of reduce trails the last landed byte.
# - Per row block, half-partition DVE reduces combine the 5 partials straight
#   into zero-masked f32r lhsT columns (col 2rb top half, 2rb+1 bottom), and
#   a single-pass fp32r matmul accumulates into PSUM on top of an early bias
#   matmul (ones/bias rows come from the same packed param tensor).
# - The packed param tensor rides the sync ring IN-LINE mid-stream: in-ring
#   it costs only its bytes, while a third queue of small descriptors steals
#   whole engine turns from the packet round-robin. x DMA creation order
#   keeps semaphore-lane reuse same-ring and far apart — a lane-reuse wait
#   is a FIFO stall for every descriptor behind it.
# - exp-accumulate + log + 10x on ACT with the (Exp+Ln+Copy) table set
#   preloaded, so no ACT_TABLE_LOAD lands anywhere.

import os

import numpy as np

import concourse.bacc as bacc
import concourse.mybir as mybir
import concourse.tile as tile
from concourse.bass_utils import run_bass_kernel_spmd
from concourse.hw_specs import get_activation_tables

N, CIN, COUT, K, H, W = 32, 64, 128, 4, 64, 64
NCORES = 8
NLOC = N // NCORES          # 4 batches per core
HW = H * W                  # 4096
ROWS = NLOC * CIN           # 256 rows (n,ci) per core
RBLK = ROWS // 128          # 2 row blocks of 128 partitions
CHUNK = 1024                # 4 KiB descriptors
NCH = HW // CHUNK           # 4 chunks per row block: [A0, D0, D1, D2]
DSPLIT = 640                # final chunk: DVE reduces [0:640), ACT [640:1024)
NPART = 5                   # partials per row block: A0, D0, D1, D2a, D2b
SCALE = 1.0 / float((H + K - 1) * (W + K - 1))   # 1/4489

# packed param layout (one [128, PCOLS] f32r tensor):
#   [0:COUT)            wdup     - scaled tap-sums, duplicated on both halves
#   [COUT:COUT+8)       s2m      - zeroed masked lhsT groups (2 x [128,4])
#   [COUT+8:COUT+136)   biasrows - rows 0/1 = conv_bias/extra_bias (cols pad 0)
#   [COUT+136:COUT+140) ones     - rows 0/1 = 1.0 (bias matmul lhsT)
OC_S2M = COUT
OC_BIAS = COUT + RBLK * NLOC
OC_ONES = OC_BIAS + COUT
PCOLS = OC_ONES + NLOC

F32 = mybir.dt.float32
F32R = mybir.dt.float32r

_CACHE: dict = {}


def _build_module() -> bacc.Bacc:
    nc = bacc.Bacc("TRN2", target_bir_lowering=False, enable_partition_id=False)

    x_d = nc.dram_tensor("xc", [ROWS, HW], F32, kind="ExternalInput").ap()
    p_d = nc.dram_tensor("pk", [128, PCOLS], F32R, kind="ExternalInput").ap()
    y_d = nc.dram_tensor("y", [NLOC, 1], F32, kind="ExternalOutput").ap()

    with tile.TileContext(nc) as tc:
        with (
            tc.tile_pool(name="xpool", bufs=1) as xpool,
            tc.tile_pool(name="spool", bufs=2) as spool,
            tc.tile_pool(name="small", bufs=1) as small,
            tc.tile_pool(name="psum", bufs=1, space="PSUM") as psum_pool,
        ):
            # preload the one ACT table set that covers Exp, Ln AND Copy
            # ("natural_log_exp_and_others") so no ACT_TABLE_LOAD is inserted
            # anywhere in the chain.
            act_tables = get_activation_tables(nc.m.arch)
            set_id = next(
                i
                for i, (_, funcs) in enumerate(act_tables.items())
                if mybir.ActivationFunctionType.Exp in funcs
                and mybir.ActivationFunctionType.Ln in funcs
                and mybir.ActivationFunctionType.Copy in funcs
            )
            nc.scalar.add_instruction(
                mybir.InstLoadActFuncSet(
                    name=nc.get_next_instruction_name(), act_func_set_id=set_id
                )
            )

            parts = small.tile([128, RBLK * NPART], F32)
            param = small.tile([128, PCOLS], F32R)
            scratch = [
                spool.tile([128, CHUNK], F32, name=f"scratch{j}")
                for j in range(2)
            ]

            wdup = param[:, 0:COUT]
            s2m = param[:, OC_S2M:OC_BIAS]
            biasrows = param[0:2, OC_BIAS:OC_ONES]
            onesb = param[0:2, OC_ONES:PCOLS]

            # ---- x chunk DMAs (+ params in-line on the SP ring) ----
            # per row block: A0 (ACT ring) = cols 0:1024, D0/D1/D2 (SP ring).
            # The packed param DMA rides the SP ring IN-LINE between the rb0
            # and rb1 D-chunks: in-ring it costs only its 0.37us of stream
    	    # time and lands right before the rb0 combine needs it, while a
            # third queue of 1KB descriptors would steal whole engine turns.
            # Creation order gives the first 8 DMAs fresh lanes; later ones
            # reuse lanes whose previous owner completes long before the
            # ring's FIFO reaches them.
            AC = {0: (0,), 1: (0,)}        # ACT-ring chunk ids per row block
            order = [
                (0, 1), (0, 0), (0, 2), (0, 3),
                (1, 1), (1, 0), (1, 2), (1, 3),
            ]
            xts = {}
            for rb, c in order:
                if (rb, c) == (1, 1):
                    # param rides the sync ring in-line between the rb0 and
                    # rb1 D-chunks: in-ring it costs only its 0.4us of stream
                    # time and lands just before the rb0 combine needs it
                    nc.sync.dma_start(out=param, in_=p_d)
                xt = xpool.tile([128, CHUNK], F32, tag=f"xt{rb}_{c}")
                eng = nc.scalar if c in AC[rb] else nc.sync
                eng.dma_start(
                    out=xt,
                    in_=x_d[rb * 128 : (rb + 1) * 128, c * CHUNK : (c + 1) * CHUNK],
                )
                xts[(rb, c)] = xt

            # ---- early bias matmul opens the f32r PSUM accumulation group ----
            pooled = psum_pool.tile([NLOC, COUT], F32, space="PSUM")
            nc.tensor.matmul(
                out=pooled, lhsT=onesb, rhs=biasrows, start=True, stop=False
            )

            # ---- per-chunk partial sums + per-row-block masked matmul ----
            with nc.allow_low_precision(
                reason="f32r combine outputs are 32-bit storage; only the PE "
                "multiply rounds, and rel-err budget is 2e-2"
            ):
                def chunk_reductions(rb):
                    pc = rb * NPART
                    nc.scalar.activation(
                        out=scratch[0],
                        in_=xts[(rb, 0)],
                        func=mybir.ActivationFunctionType.Copy,
                        accum_out=parts[:, pc : pc + 1],
                    )
                    for j, c in enumerate((1, 2)):
                        nc.vector.reduce_sum(
                            out=parts[:, pc + 1 + j : pc + 2 + j],
                            in_=xts[(rb, c)],
                            axis=mybir.AxisListType.X,
                        )
                    # trailing split: DVE takes [0:DSPLIT), idle ACT the rest
                    xl = xts[(rb, 3)]
                    nc.vector.reduce_sum(
                        out=parts[:, pc + 3 : pc + 4],
                        in_=xl[:, 0:DSPLIT],
                        axis=mybir.AxisListType.X,
                    )
                    nc.scalar.activation(
                        out=scratch[1][:, 0 : CHUNK - DSPLIT],
                        in_=xl[:, DSPLIT:CHUNK],
                        func=mybir.ActivationFunctionType.Copy,
                        accum_out=parts[:, pc + 4 : pc + 5],
                    )

                def combine_and_matmul(rb):
                    # combine all 5 partials straight into the masked f32r
                    # halves (half-partition reduces cost only ~170ns each)
                    pc = rb * NPART
                    g = 4 * rb  # s2m-local group base
                    nc.vector.reduce_sum(
                        out=s2m[0:64, g + 2 * rb : g + 2 * rb + 1],
                        in_=parts[0:64, pc : pc + NPART],
                        axis=mybir.AxisListType.X,
                    )
                    nc.vector.reduce_sum(
                        out=s2m[64:128, g + 2 * rb + 1 : g + 2 * rb + 2],
                        in_=parts[64:128, pc : pc + NPART],
                        axis=mybir.AxisListType.X,
                    )
                    nc.tensor.matmul(
                        out=pooled,
                        lhsT=s2m[:, g : g + NLOC],
                        rhs=wdup,
                        start=False,
                        stop=(rb == RBLK - 1),
                        skip_group_check=True,
                    )

                chunk_reductions(0)
                combine_and_matmul(0)
                chunk_reductions(1)
                combine_and_matmul(1)

            # ---- 10 * log(sum_co exp(pooled)) ----
            expt = small.tile([NLOC, COUT], F32)
            sume = small.tile([NLOC, 1], F32)
            nc.scalar.activation(
                out=expt,
                in_=pooled,
                func=mybir.ActivationFunctionType.Exp,
                accum_out=sume,
            )
            logv = small.tile([NLOC, 1], F32)
            nc.scalar.activation(
                out=logv, in_=sume, func=mybir.ActivationFunctionType.Ln
            )
            outv = small.tile([NLOC, 1], F32)
            nc.scalar.mul(out=outv, in_=logv, mul=10.0)
            nc.sync.dma_start(out=y_d, in_=outv)

    nc.compile()
    return nc


def kernel(x, weight, conv_bias, extra_bias):
    x = np.ascontiguousarray(np.asarray(x, dtype=np.float32))
    weight = np.ascontiguousarray(np.asarray(weight, dtype=np.float32))
    conv_bias = np.ascontiguousarray(np.asarray(conv_bias, dtype=np.float32))
    extra_bias = np.ascontiguousarray(np.asarray(extra_bias, dtype=np.float32))
    assert x.shape == (N, CIN, H, W), x.shape
    assert weight.shape == (CIN, COUT, K, K), weight.shape

    if "nc" not in _CACHE:
        _CACHE["nc"] = _build_module()
    nc = _CACHE["nc"]

    # host-side param packing: scaled weight tap-sums duplicated onto both
    # partition halves | zeroed mask groups | bias rows | ones rows.
    ws = (weight.reshape(CIN, COUT, K * K).sum(axis=2) * SCALE).astype(np.float32)
    pk = np.zeros((128, PCOLS), dtype=np.float32)
    pk[0:CIN, 0:COUT] = ws
    pk[CIN:128, 0:COUT] = ws
    pk[0, OC_BIAS:OC_ONES] = conv_bias
    pk[1, OC_BIAS:OC_ONES] = extra_bias
    pk[0:2, OC_ONES:PCOLS] = 1.0
    pk = np.ascontiguousarray(pk)
    in_maps = []
    for c in range(NCORES):
        xc = x[c * NLOC : (c + 1) * NLOC].reshape(ROWS, HW)
        in_maps.append({"xc": xc, "pk": pk})

    trace = os.environ.get("BASS_KERNEL_TRACE") == "1"
    res = run_bass_kernel_spmd(
        nc, in_maps, core_ids=list(range(NCORES)), trace=trace
    )
    _CACHE["last_result"] = res
    return np.concatenate([r["y"] for r in res.results], axis=0)
